# revision 37
# baseline (speedup 1.0000x reference)
"""Multi-head attention (QKV proj + rotary + softmax attention + out proj)
for Trainium2, sharded over 8 NeuronCores.

Problem: x[2,2048,1024], 16 heads x dh=64, rotary embedding, softmax
attention, output projection + bias.

Sharding: batch x head-group. Core c handles batch c//4 and the 4 heads
[4*(c%4), 4*(c%4)+4). Each core computes its QKV slice, rotary, attention,
and a partial output projection; the host sums the 4 partial projections
per batch and adds the bias.

Device-side design (per core, everything in "transposed" layout):
  - all matmul operands are fp16 (PE runs fp16 at 1 col/cycle vs the
    2 cyc/col fp32_mode=HIGH path that f32r lowers to); accumulation
    stays fp32 in PSUM so precision is set by the 10-bit fp16 mantissa
    of the operands only.
  - qkvT = W @ x^T: qT/kT produced as [dh-pair(128), n] tiles, v as
    natural [n, e] tiles.
  - rotary applied on the fp32 psum output via DVE: q*cos +
    pairswap(q*sin_pre), with the dh dimension stored interleaved
    ([0,32,1,33,...]) so rotate_half becomes an adjacent-lane
    stream_shuffle. Output fp16.
  - dots: scoresT[j,n] = krotT^T-slice @ qrotT, two heads packed in the
    128x128 PE array via tile_position row-tiling (K=64 each). fp32 psum.
  - softmax without max-subtraction (logits are O(+-6)): ACT exp over
    2-j-tile psum batches (N=1024 per ACTIVATE), output fp16.
  - AV: lhsT = [v | ones] (M=65, fp16) so row 64 accumulates the softmax
    denominators for free; fp32 psum accumulation over the 16 j-tiles.
  - normalize: reciprocal_approx_fast of the sums row, partition-broadcast
    via a DRAM round-trip DMA (K=1 ones-matmul on the tail block), one DVE
    multiply -> aoT (fp16).
  - output proj: y[n,d] accumulated over the two head-pair e-chunks, fp16
    out; the last block's pair-0 partial goes to a separate output (y3a)
    summed on the host, so the kernel tail only runs the pair-1 projection.
  - DMA: weights/x prepped host-side into the exact SBUF layouts so every
    load is contiguous; loads are split across the two HWDGE rings (sync:
    wqk + x + recip round-trips, scalar: rotary tables + wv + wo + y
    stores) so x tiles don't queue behind the constant pool.
"""
import sys

sys.path.insert(0, "/opt/trn_rl_repo")

import numpy as np

import concourse.bacc as bacc
import concourse.tile as tile
from concourse import mybir
from concourse.bass_utils import run_bass_kernel_spmd

F32 = mybir.dt.float32
FP16 = mybir.dt.float16
EXP = mybir.ActivationFunctionType.Exp
MULT = mybir.AluOpType.mult
ADD = mybir.AluOpType.add

B, N, DIM = 2, 2048, 1024
H, DH = 16, 64
INNER = H * DH
SCALE = DH ** -0.5
NCORES = 8
HPC = H // (NCORES // B)      # heads per core = 4
NPAIR = HPC // 2              # head pairs per core = 2

P = 128
NT = N // 512                 # 4 n-tiles of 512
DC = DIM // P                 # 8 d-chunks
JTILES = N // P               # 16 j-tiles
JB = JTILES // 2              # 8 j-batches (2 j-tiles each)

PAIRSWAP = [i ^ 1 for i in range(32)]

_CACHE = {}


def _build():
    nc = bacc.Bacc(None, target_bir_lowering=False, debug=False)
    with tile.TileContext(nc) as tc:
        with tc.tile_pool(name="dram", bufs=1, space="DRAM") as dram, \
             tc.tile_pool(name="const", bufs=1) as const, \
             tc.tile_pool(name="perst", bufs=1) as perst, \
             tc.tile_pool(name="tmp", bufs=1) as tmp, \
             tc.tile_pool(name="ps", bufs=1, space="PSUM") as ps:
            # ---------------- DRAM I/O (all host-prearranged, contiguous) ---
            xT_d = dram.tile([P, NT * DC * 512], FP16, kind="ExternalInput", name="xh", uniquify=False)
            wqk_d = dram.tile([P, 4 * DC * P], FP16, kind="ExternalInput", name="wqkh", uniquify=False)
            wv_d = dram.tile([P, DC * 256], FP16, kind="ExternalInput", name="wvh", uniquify=False)
            wo_d = dram.tile([P, NPAIR * DIM], FP16, kind="ExternalInput", name="woh", uniquify=False)
            csq_d = dram.tile([P, 2 * N], FP16, kind="ExternalInput", name="csq", uniquify=False)
            csk_d = dram.tile([P, 2 * N], FP16, kind="ExternalInput", name="csk", uniquify=False)
            y_d = dram.tile([N, DIM], FP16, kind="ExternalOutput", name="y", uniquify=False)
            y3a_d = dram.tile([512, DIM], FP16, kind="ExternalOutput", name="y3a", uniquify=False)

            # ---------------- constants to SBUF ----------------
            # sync ring: wqk (k0 first, matching first-use order); scalar
            # ring: rotary tables + wv + wo, also in first-use order.
            wqk_r = wqk_d.rearrange("p (a c e) -> p a c e", a=4, c=DC)
            wqk_sb = [const.tile([P, DC, P], FP16, name=f"wqk{ech}")
                      for ech in range(4)]
            # k0's weights load first; x tile 0 second (see load_x below);
            # the other three wqk chunks follow on the sync ring
            nc.sync.dma_start(wqk_sb[2][:, :, :], wqk_r[:, 2, :, :])
            csk_sb = const.tile([P, 2, N], FP16)
            nc.scalar.dma_start(csk_sb[:, :, :], csk_d.rearrange("p (a n) -> p a n", a=2))
            csq_sb = const.tile([P, 2, N], FP16)
            nc.scalar.dma_start(csq_sb[:, :, :], csq_d.rearrange("p (a n) -> p a n", a=2))
            ck_sb, sk_sb = csk_sb[:, 0, :], csk_sb[:, 1, :]
            cq_sb, sq_sb = csq_sb[:, 0, :], csq_sb[:, 1, :]
            # wv/wo tiles declared here, loads deferred into the emission
            # stream (wv first needed mid-first-attention, wo only at nq=1
            # jb=4) to keep startup SDMA bandwidth for wqk/x/rotary tables
            wv_sb = const.tile([P, DC, 256], FP16)
            wo_sb = const.tile([P, NPAIR, DIM], FP16)

            def load_wv():
                nc.scalar.dma_start(wv_sb[:, :, :],
                                    wv_d.rearrange("p (c e) -> p c e", c=DC))

            def load_wo():
                nc.scalar.dma_start(wo_sb[:, :, :],
                                    wo_d.rearrange("p (a d) -> p a d", a=NPAIR))

            ones_f = const.tile([1, 64], F32)
            nc.vector.memset(ones_f[:, :], 1.0)
            ones_h = const.tile([1, 64], FP16)
            nc.vector.tensor_copy(ones_h[:, :], ones_f[:, :])
            nbias = const.tile([P, 1], F32)
            nc.vector.memset(nbias[:, :], -3.0)

            # ---------------- persistent tiles ----------------
            qrot = [[perst.tile([P, 512], FP16, name=f"qrot{p}_{t}")
                     for t in range(NT)] for p in range(NPAIR)]
            krot = [[perst.tile([P, 512], FP16, name=f"krot{p}_{t}")
                     for t in range(NT)] for p in range(NPAIR)]
            v_aug = [perst.tile([P, 4, HPC, 65], FP16, name=f"vaug{t}")
                     for t in range(NT)]
            for t in range(NT):
                nc.vector.memset(v_aug[t][:, :, :, 64:65], 1.0)
            aoT = [[perst.tile([P, 512], FP16, name=f"aoT{p}_{t}")
                    for t in range(NT)] for p in range(NPAIR)]

            # ---------------- helpers ----------------
            xT_r = xT_d.rearrange("p (t c n) -> p t c n", t=NT, c=DC)

            xts = [None] * NT

            def load_x(t):
                # persistent per-tile x: one contiguous 1MB DMA on first use,
                # reused by every later qkv pass over the same tile
                if xts[t] is None:
                    xt = perst.tile([P, DC, 512], FP16, name=f"x{t}")
                    nc.sync.dma_start(xt[:, :, :], xT_r[:, t, :, :])
                    xts[t] = xt
                    if t == 0:
                        nc.sync.dma_start(wqk_sb[0][:, :, :], wqk_r[:, 0, :, :])
                    elif t == 1:
                        for ech in (3, 1):
                            nc.sync.dma_start(wqk_sb[ech][:, :, :],
                                              wqk_r[:, ech, :, :])
                return xts[t]

            def qk_chunk(ech, t, xt, dest, cos_sb, sin_sb):
                # qkvT e-chunk [128, 512] = W-chunk @ xT-tile, then rotary.
                pqk = ps.tile([P, 512], F32, name="pqk", tag="m", bufs=2)
                for c in range(DC):
                    nc.tensor.matmul(pqk[:, :],
                                     wqk_sb[ech][:, c, :],
                                     xt[:, c, :],
                                     start=(c == 0), stop=(c == DC - 1))
                sl = slice(t * 512, (t + 1) * 512)
                t1 = tmp.tile([P, 512], FP16, name="t1", tag="t1", bufs=2)
                t2 = tmp.tile([P, 512], FP16, name="t2", tag="t2", bufs=2)
                t3 = tmp.tile([P, 512], FP16, name="t3", tag="t3", bufs=2)
                nc.vector.tensor_tensor(t1[:, :], pqk[:, :], cos_sb[:, sl], op=MULT)
                nc.vector.tensor_tensor(t2[:, :], pqk[:, :], sin_sb[:, sl], op=MULT)
                nc.vector.stream_shuffle(t3[:, :], t2[:, :], PAIRSWAP)
                nc.vector.tensor_tensor(dest[:, :], t1[:, :], t3[:, :], op=ADD)

            def v_tile(t, xt, subs=range(4)):
                # v natural [n, e] for the 4 local heads, by 128-row subtiles
                for nsub in subs:
                    pv = ps.tile([P, 256], F32, name="pv", tag="m", bufs=2)
                    for c in range(DC):
                        nc.tensor.matmul(pv[:, :],
                                         xt[:, c, nsub * P:(nsub + 1) * P],
                                         wv_sb[:, c, :],
                                         start=(c == 0), stop=(c == DC - 1))
                    nc.vector.tensor_copy(
                        v_aug[t][:, nsub, :, 0:64],
                        pv[:, :].rearrange("p (h d) -> p h d", h=HPC))

            def qkv_for_tile(t, ops):
                xt = load_x(t)
                for op in ops:
                    if op == "k0":
                        qk_chunk(2, t, xt, krot[0][t], ck_sb, sk_sb)
                    elif op == "k1":
                        qk_chunk(3, t, xt, krot[1][t], ck_sb, sk_sb)
                    elif op == "q0":
                        qk_chunk(0, t, xt, qrot[0][t], cq_sb, sq_sb)
                    elif op == "q1":
                        qk_chunk(1, t, xt, qrot[1][t], cq_sb, sq_sb)
                    elif op == "v":
                        v_tile(t, xt)
                    elif op == "va":
                        v_tile(t, xt, subs=(0, 1))
                    elif op == "vb":
                        v_tile(t, xt, subs=(2, 3))

            # software pipelining across blocks: each block's last-jb AV and
            # its psum evacuation are deferred until just after the NEXT
            # block's first scores+exp have been emitted, so the ACT engine
            # crosses block boundaries without a gap
            pending_fin = [None]

            def attention(nq, pair, pre_jb=None, mid_jb=None, tail=False,
                          defer=False):
                pav = [ps.tile([65, 512], F32, name=f"pav{h}", tag="av", bufs=2)
                       for h in range(2)]
                deferred = []
                for jb in range(JB):
                    if pre_jb is not None:
                        pre_jb(jb)
                    # per-jl tiles with the two heads adjacent: consecutive
                    # score matmuls then alternate PE row-groups (h0/h64) and
                    # write different psum banks, the layout concurrency needs
                    sc = [ps.tile([P, 2, 512], F32, name=f"sc{jl}", tag="s", bufs=2)
                          for jl in range(2)]
                    for jl in range(2):
                        jt = jb * 2 + jl
                        kt = krot[pair][jt // 4]
                        jsl = slice((jt % 4) * P, (jt % 4 + 1) * P)
                        for h in range(2):
                            rows = slice(h * 64, (h + 1) * 64)
                            nc.tensor.matmul(sc[jl][:, h, :],
                                             kt[rows, jsl],
                                             qrot[pair][nq][rows, :],
                                             start=True, stop=True,
                                             tile_position=(h * 64, 0))
                    ex = [tmp.tile([P, 2, 512], FP16, name=f"ex{jl}", tag="ex", bufs=4)
                          for jl in range(2)]
                    for jl in range(2):
                        # bias -3 keeps exp sums and the un-normalized AV psum
                        # inside fp16 range (max logit ~10.5); the denominator
                        # picks up the same factor so softmax is unchanged
                        nc.scalar.activation(ex[jl][:, :, :], sc[jl][:, :, :],
                                             EXP, bias=nbias[:, :])
                    if mid_jb is not None:
                        mid_jb(jb)
                    if jb == 0 and pending_fin[0] is not None:
                        fin = pending_fin[0]
                        pending_fin[0] = None
                        fin()
                    for jl in range(2):
                        jt = jb * 2 + jl
                        if defer and jb == JB - 1:
                            deferred.append((jt, jl))
                            continue
                        for h in range(2):
                            nc.tensor.matmul(pav[h][:, :],
                                             v_aug[jt // 4][:, jt % 4, pair * 2 + h, :],
                                             ex[jl][:, h, :],
                                             start=(jt == 0), stop=(jt == JTILES - 1))
                    last_ex = ex
                if defer:
                    def fin(last_ex=last_ex, pav=pav, nq=nq, pair=pair,
                            deferred=deferred, tail=tail):
                        for jt, jl in deferred:
                            for h in range(2):
                                nc.tensor.matmul(
                                    pav[h][:, :],
                                    v_aug[jt // 4][:, jt % 4, pair * 2 + h, :],
                                    last_ex[jl][:, h, :],
                                    start=(jt == 0), stop=(jt == JTILES - 1))
                        _evac(nq, pair, pav, tail)
                    pending_fin[0] = fin
                else:
                    _evac(nq, pair, pav, tail)

            def _evac(nq, pair, pav, tail):
                for h in range(2):
                    # evacuate psum immediately so the next (nq, pair) can start;
                    # sums row copied separately so it lands at partition 0
                    # (custom-DVE reciprocal_approx_fast requires base_partition 0)
                    av_sb = tmp.tile([64, 512], FP16, name="av_sb", tag="avs", bufs=3)
                    sm_sb = tmp.tile([1, 512], F32, name="sm_sb", tag="sms", bufs=4)
                    # sums row first: the recip -> broadcast -> multiply chain
                    # is the long pole at block boundaries. In the very last
                    # block the exp stream is over, so the idle ACT engine
                    # takes the copies and halves the serial tail chain.
                    cp = nc.scalar.copy if tail else nc.vector.tensor_copy
                    cp(sm_sb[:, :], pav[h][64:65, :])
                    cp(av_sb[:, :], pav[h][0:64, :])
                    rc = tmp.tile([1, 512], F32, name="rc", tag="rc", bufs=2)
                    nc.vector.reciprocal_approx_fast(rc[:, :], sm_sb[:, :])
                    rch = tmp.tile([1, 512], FP16, name="rch", tag="rch", bufs=2)
                    nc.vector.tensor_copy(rch[:, :], rc[:, :])
                    rows = slice(h * 64, (h + 1) * 64)
                    if nq == NT - 1:
                        # tail-critical: broadcast via K=1 ones-matmul (no DMA
                        # round-trip latency before the last y projection);
                        # multiply straight from psum to skip a CAST
                        pbc = ps.tile([64, 512], F32, name="pbc", tag="m", bufs=2)
                        nc.tensor.matmul(pbc[:, :], ones_h[:, :], rch[:, :],
                                         start=True, stop=True)
                        nc.vector.tensor_tensor(aoT[pair][nq][rows, :],
                                                av_sb[:, :], pbc[:, :], op=MULT)
                    else:
                        # broadcast across partitions via a DRAM round-trip
                        bc = tmp.tile([64, 512], FP16, name="bc", tag="bc", bufs=2)
                        rd = dram.tile([1, 512], FP16, name="rd", tag="rd", bufs=2)
                        nc.sync.dma_start(rd[:, :], rch[:, :])
                        nc.sync.dma_start(bc[:, :], rd.to_broadcast([64, 512]))
                        nc.vector.tensor_tensor(aoT[pair][nq][rows, :],
                                                av_sb[:, :], bc[:, :], op=MULT)

            def y_proj_pair_sub(nq, pair, out_d, row0, nsub):
                # one 128-row slice of a single-pair partial projection;
                # stores alternate between the two HWDGE rings so the final
                # 1MB of output drains in parallel
                ys = tmp.tile([P, DIM], FP16, name="ysp", tag="ys", bufs=2)
                nsl = slice(nsub * P, (nsub + 1) * P)
                for dh2 in range(2):
                    py = ps.tile([P, 512], F32, name="pyp", tag="m", bufs=2)
                    dsl = slice(dh2 * 512, (dh2 + 1) * 512)
                    nc.tensor.matmul(py[:, :], aoT[pair][nq][:, nsl],
                                     wo_sb[:, pair, dsl],
                                     start=True, stop=True)
                    nc.vector.tensor_copy(ys[:, dsl], py[:, :])
                ring = nc.scalar if nsub % 2 == 0 else nc.sync
                ring.dma_start(out_d[row0 + nsub * P:row0 + (nsub + 1) * P, :],
                               ys[:, :])

            def y_proj_pair(nq, pair, out_d, row0):
                for nsub in range(4):
                    y_proj_pair_sub(nq, pair, out_d, row0, nsub)

            def y_proj_sub(nq, nsub):
                # one 128-row slice of the full (both-pair) projection
                ys = tmp.tile([P, DIM], FP16, name="ys", tag="ys", bufs=2)
                nsl = slice(nsub * P, (nsub + 1) * P)
                for dh2 in range(2):
                    py = ps.tile([P, 512], F32, name="py", tag="m", bufs=2)
                    dsl = slice(dh2 * 512, (dh2 + 1) * 512)
                    for pair in range(NPAIR):
                        nc.tensor.matmul(py[:, :],
                                         aoT[pair][nq][:, nsl],
                                         wo_sb[:, pair, dsl],
                                         start=(pair == 0), stop=(pair == NPAIR - 1))
                    nc.vector.tensor_copy(ys[:, dsl], py[:, :])
                ring = nc.scalar if nsub % 2 == 0 else nc.sync
                ring.dma_start(y_d[nq * 512 + nsub * P:
                                   nq * 512 + (nsub + 1) * P, :], ys[:, :])

            # ---------------- emission order ----------------
            # Tile has sequential program-order semantics: every tile must be
            # written (in emission order) before anything that reads it, and
            # per-psum-tag slot reuse is FIFO in emission order.
            #
            # The wall-clock floor of this kernel is (first exp) + 128
            # ACTIVATEs + tail, so the exp stream must never starve: each
            # jb's scores are emitted first, and all other PE work (qkv
            # chunks for later tiles, output projections) is threaded into
            # the mid hooks that sit AFTER the two ACTIVATE emissions of the
            # jb, i.e. it executes inside the ~2.2us exp shadow instead of
            # ahead of the next scores.
            # Blocks run pair-outer -- (0,0),(1,0),(2,0),(3,0),(0,1),...,(3,1)
            # -- so pair 1's k/q preparation spreads over four extra exp
            # shadows instead of piling into the first two blocks. v and k0
            # (consumed by the very first block) still front-load; v0 fills
            # the otherwise-idle PE window before the first scores.
            load_wv()
            qkv_for_tile(0, ["k0", "q0"])
            # tile 1's k and first v-half run in the PE-idle window while
            # DVE finishes the t0 rotary chains (before the first scores)
            qkv_for_tile(1, ["k0", "va"])
            load_x(2)
            load_x(3)

            def mid_b1(jb):          # (0,0)
                if jb == 0:
                    qkv_for_tile(0, ["v"])
                elif jb == 1:
                    qkv_for_tile(1, ["vb"])
                elif jb == 2:
                    qkv_for_tile(2, ["k0"])
                elif jb == 3:
                    qkv_for_tile(2, ["va"])
                elif jb == 4:
                    qkv_for_tile(2, ["vb"])
                elif jb == 5:
                    qkv_for_tile(3, ["k0", "va"])
                elif jb == 6:
                    qkv_for_tile(3, ["vb"])
                    qkv_for_tile(1, ["q0"])

            def mid_b2(jb):          # (1,0)
                if jb == 0:
                    qkv_for_tile(0, ["k1"])
                elif jb == 1:
                    qkv_for_tile(2, ["q0"])
                    load_wo()
                elif jb == 3:
                    qkv_for_tile(1, ["k1"])
                elif jb == 5:
                    qkv_for_tile(0, ["q1"])

            def mid_b3(jb):          # (2,0)
                if jb == 1:
                    qkv_for_tile(3, ["q0"])
                elif jb == 3:
                    qkv_for_tile(2, ["k1"])
                elif jb == 5:
                    qkv_for_tile(1, ["q1"])

            def mid_b4(jb):          # (3,0)
                if jb == 1:
                    qkv_for_tile(3, ["k1"])
                elif jb == 3:
                    qkv_for_tile(2, ["q1"])
                elif jb == 5:
                    qkv_for_tile(3, ["q1"])

            def mid_b5(jb):          # (0,1): pair-0 part of tile 3's y
                if 2 <= jb <= 5:
                    y_proj_pair_sub(NT - 1, 0, y3a_d, 0, jb - 2)

            def make_mid_y(nq, jb0):
                def mid(jb):
                    if jb0 <= jb < jb0 + 4:
                        y_proj_sub(nq, jb - jb0)
                return mid

            order = [(0, 0), (1, 0), (2, 0), (3, 0),
                     (0, 1), (1, 1), (2, 1), (3, 1)]
            mids = [mid_b1, mid_b2, mid_b3, mid_b4,
                    mid_b5, make_mid_y(0, 4), make_mid_y(1, 4),
                    make_mid_y(2, 1)]
            for bi, ((nq, pair), mid) in enumerate(zip(order, mids)):
                attention(nq, pair, mid_jb=mid, tail=(bi == 7),
                          defer=(bi < 7))
            y_proj_pair(NT - 1, 1, y_d, (NT - 1) * 512)
    nc.compile()
    return nc


def _host_prep(x, rotary_emb, w_qkv, w_out):
    """Build the 8 per-core input maps (everything pre-laid-out + fp16)."""
    x = np.asarray(x, dtype=np.float32)
    rotary_emb = np.asarray(rotary_emb, dtype=np.float32)
    w_qkv = np.asarray(w_qkv, dtype=np.float32)
    w_out = np.asarray(w_out, dtype=np.float32)

    # interleaved dh permutation: new row 2i <- dim i, 2i+1 <- dim 32+i
    perm = np.empty(DH, dtype=np.int64)
    perm[0::2] = np.arange(32)
    perm[1::2] = np.arange(32) + 32
    pair_swap = np.arange(DH) ^ 1

    cos = np.cos(rotary_emb).T[perm]                      # [dh, n] permuted
    sin = np.sin(rotary_emb).T[perm]
    sign = np.where(perm < 32, -1.0, 1.0)[:, None].astype(np.float32)
    sin_eff = sign * sin
    sin_pre = sin_eff[pair_swap]                          # pre-swapped
    c2 = np.concatenate([cos, cos], axis=0)               # [128, n]
    s2 = np.concatenate([sin_pre, sin_pre], axis=0)
    csq = np.ascontiguousarray(np.concatenate(
        [SCALE * c2, SCALE * s2], axis=1).astype(np.float16))   # [128, 2*N]
    csk = np.ascontiguousarray(np.concatenate(
        [c2, s2], axis=1).astype(np.float16))

    in_maps = []
    for core in range(NCORES):
        b = core // (NCORES // B)
        g = core % (NCORES // B)
        heads = range(4 * g, 4 * g + HPC)
        q_rows = np.concatenate([h * DH + perm for h in heads])
        k_rows = np.concatenate([INNER + h * DH + perm for h in heads])
        v_rows = np.arange(2 * INNER + 4 * g * DH, 2 * INNER + (4 * g + HPC) * DH)

        # wqk: [p, ech, c, e] with ech 0/1 = q pairs, 2/3 = k pairs
        Wqk = w_qkv[np.concatenate([q_rows, k_rows])]     # [512, 1024]
        wqkh = Wqk.reshape(4, P, DC, P).transpose(3, 0, 2, 1)
        wqkh = np.ascontiguousarray(
            wqkh.reshape(P, 4 * DC * P).astype(np.float16))

        # wv: [p, c, e]
        Wv = w_qkv[v_rows]                                # [256, 1024]
        wvh = Wv.reshape(256, DC, P).transpose(2, 1, 0)
        wvh = np.ascontiguousarray(
            wvh.reshape(P, DC * 256).astype(np.float16))

        # wo: [p, pair, d]
        Wo = w_out[:, 4 * g * DH:(4 * g + HPC) * DH]      # [1024, 256]
        woh = Wo.T.reshape(NPAIR, P, DIM).transpose(1, 0, 2)
        woh = np.ascontiguousarray(
            woh.reshape(P, NPAIR * DIM).astype(np.float16))

        # x: [p, t, c, n']
        xh = x[b].reshape(NT, 512, DC, P).transpose(3, 0, 2, 1)
        xh = np.ascontiguousarray(
            xh.reshape(P, NT * DC * 512).astype(np.float16))

        in_maps.append({
            "xh": xh, "wqkh": wqkh, "wvh": wvh, "woh": woh,
            "csq": csq, "csk": csk,
        })
    return in_maps


def kernel(x, rotary_emb, w_qkv, w_out, b_out, _trace=False):
    if "nc" not in _CACHE:
        _CACHE["nc"] = _build()
    nc = _CACHE["nc"]
    in_maps = _host_prep(x, rotary_emb, w_qkv, w_out)
    res = run_bass_kernel_spmd(nc, in_maps, core_ids=list(range(NCORES)),
                               trace=_trace)
    _CACHE["last_result"] = res
    y = np.zeros((B, N, DIM), dtype=np.float32)
    for core in range(NCORES):
        b = core // (NCORES // B)
        y[b] += res.results[core]["y"].astype(np.float32)
        y[b, (NT - 1) * 512:] += res.results[core]["y3a"].astype(np.float32)
    y += np.asarray(b_out, dtype=np.float32)[None, None, :]
    return y
